# revision 14
# baseline (speedup 1.0000x reference)
"""Trainium2 Bass kernel for nn_CombinedGNN (gnn_message_passing).

Strategy (8 NeuronCores, node/row parallel, zero collectives):
  - masks[1] in the reference is identically zero (elementwise pow of a 0/1
    matrix), so only mask0 = adj/rowdeg matters.
  - All T=12 timesteps' aggregations are mask0 @ data[t] -> batched into ONE
    matmul  adj @ [X | 1]  with X = data rearranged to [N, 96]; the ones
    column yields row degrees, and the 1/deg row scaling is applied after.
  - Each core owns 625 nodes (padded to 640). It gets adj^T's column block
    (so the contraction dim sits on SBUF partitions with contiguous DMA) and
    computes its nodes' full output independently.
  - The sequential t-chain (his_prev/cur_prev recurrences) runs in
    [feature-on-partition, node-on-free] orientation with host-prepacked /
    permuted weight matrices so no on-chip transposes are needed.
  - adj (exactly representable 0/1) and X are cast to bf16 for the big
    matmul; accumulation is fp32 in PSUM. Everything downstream is fp32.
"""

import numpy as np
import ml_dtypes

import concourse.bass as bass
import concourse.mybir as mybir
import concourse.bass_utils as bass_utils
from concourse.tile import TileContext

# problem constants (hardcoded per harness contract)
N, T, DAY, L = 5000, 12, 8, 2
F = DAY - 1
DIM = T * DAY  # 96
NCORES = 8
NPC = N // NCORES        # 625 nodes per core
NP = 640                 # padded nodes per core
NH = NP // 2             # 320, node half processed per psum chunk
KT = 125                 # contraction tile (partitions)
NKT = N // KT            # 40
KG = 8                   # k-tiles per DMA group
NG = NKT // KG           # 5
XW = DIM + 1             # 97: 96 features + ones column

F32 = mybir.dt.float32
BF16 = mybir.dt.bfloat16
BF16_NP = ml_dtypes.bfloat16

_MAXW = 1


def split_multi_waits(nc):
    """Walrus in this container rejects instructions with >~2 sync waits.
    Hoist extra waits onto preceding single-wait NoOps on the same engine."""
    f = nc.m.functions[0]
    for bb in list(f.blocks):
        new, ctr = [], 0
        for inst in bb.instructions:
            si = inst.sync_info
            waits = list(si.on_wait) if (si and si.on_wait) else []
            if len(waits) > _MAXW:
                head, keep = waits[:-_MAXW], waits[-_MAXW:]
                for i in range(0, len(head), _MAXW):
                    nop = mybir.InstNoOp(
                        name=f"{inst.name}-wsplit{ctr}", engine=inst.engine,
                        ins=[], outs=[],
                        sync_info=mybir.SyncInfo(on_wait=head[i:i + _MAXW],
                                                 on_update=[]),
                    )
                    ctr += 1
                    new.append(nop)
                inst.sync_info = mybir.SyncInfo(
                    on_wait=keep,
                    on_update=list(si.on_update) if si.on_update else [])
            new.append(inst)
        bb.instructions = new


def build_nc():
    nc = bass.Bass()
    a_d = nc.dram_tensor("a", [2, KT, NKT, NH], BF16, kind="ExternalInput")
    xe_d = nc.dram_tensor("xe", [KT, NKT, XW], BF16, kind="ExternalInput")
    dt_d = nc.dram_tensor("dt", [DIM, NP], F32, kind="ExternalInput")
    pt_d = nc.dram_tensor("pt", [8, T, NP], F32, kind="ExternalInput")
    wraw_d = nc.dram_tensor("wraw", [DIM, DIM], F32, kind="ExternalInput")
    wagg_d = nc.dram_tensor("wagg", [DIM, DIM], F32, kind="ExternalInput")
    wprev_d = nc.dram_tensor("wprev", [8, DIM], F32, kind="ExternalInput")
    w2_d = nc.dram_tensor("w2", [8, DIM], F32, kind="ExternalInput")
    wf_d = nc.dram_tensor("wf", [8, T * DIM], F32, kind="ExternalInput")
    out_d = nc.dram_tensor("out", [DIM, NP], F32, kind="ExternalOutput")

    with TileContext(nc) as tc:
        with (
            tc.tile_pool(name="const", bufs=1) as cpool,
            tc.tile_pool(name="adma", bufs=3) as apool,
            tc.tile_pool(name="work", bufs=2) as wpool,
            tc.tile_pool(name="pagg", bufs=1, space="PSUM") as pagg,
            tc.tile_pool(name="pbc", bufs=1, space="PSUM") as pbc,
            tc.tile_pool(name="pp1", bufs=2, space="PSUM") as pp1,
            tc.tile_pool(name="pp2", bufs=2, space="PSUM") as pp2,
            tc.tile_pool(name="ppf", bufs=2, space="PSUM") as ppf,
        ):
            # constants / persistent state
            xe_t = cpool.tile([KT, NKT, XW], BF16)
            nc.sync.dma_start(out=xe_t, in_=xe_d[:, :, :])
            dt_t = cpool.tile([DIM, NP], F32)
            nc.sync.dma_start(out=dt_t, in_=dt_d[:, :])
            pt_t = cpool.tile([8, T, NP], F32)
            nc.sync.dma_start(out=pt_t, in_=pt_d[:, :, :])
            wraw_t = cpool.tile([DIM, DIM], F32)
            nc.sync.dma_start(out=wraw_t, in_=wraw_d[:, :])
            wagg_t = cpool.tile([DIM, DIM], F32)
            nc.sync.dma_start(out=wagg_t, in_=wagg_d[:, :])
            wprev_t = cpool.tile([8, DIM], F32)
            nc.sync.dma_start(out=wprev_t, in_=wprev_d[:, :])
            w2_t = cpool.tile([8, DIM], F32)
            nc.sync.dma_start(out=w2_t, in_=w2_d[:, :])
            wf_t = cpool.tile([8, T * DIM], F32)
            nc.sync.dma_start(out=wf_t, in_=wf_d[:, :])

            ones_t = cpool.tile([1, DIM], F32)
            nc.vector.memset(ones_t, 1.0)
            h2_t = cpool.tile([8, T, NP], F32)
            prev_t = cpool.tile([8, NP], F32)
            nc.vector.memset(prev_t, 0.0)
            aggs_t = cpool.tile([DIM, NP], F32)
            outt_t = cpool.tile([DIM, NP], F32)

            # phase 1: aggT[97, NH] = [X|1]^T @ adjT_shard  per node half
            for h in range(2):
                cs = slice(h * NH, (h + 1) * NH)
                aggp = pagg.tile([XW, NH], F32, tag="aggp")
                for g in range(NG):
                    a_t = apool.tile([KT, KG, NH], BF16, tag="a")
                    nc.sync.dma_start(
                        out=a_t, in_=a_d[h, :, g * KG:(g + 1) * KG, :])
                    for j in range(KG):
                        k = g * KG + j
                        nc.tensor.matmul(aggp, xe_t[:, k, :], a_t[:, j, :],
                                         start=(k == 0), stop=(k == NKT - 1))
                # row-normalization: recip = 1/max(deg,1), broadcast, scale
                degm = wpool.tile([1, NH], F32, tag="degm")
                nc.vector.tensor_scalar_max(degm, aggp[DIM:DIM + 1, :], 1.0)
                recip = wpool.tile([1, NH], F32, tag="recip")
                nc.vector.reciprocal(recip, degm)
                rb_p = pbc.tile([DIM, NH], F32, tag="rbp")
                nc.tensor.matmul(rb_p, ones_t, recip, start=True, stop=True)
                rb_s = wpool.tile([DIM, NH], F32, tag="rbs")
                nc.vector.tensor_copy(rb_s, rb_p)
                nc.vector.tensor_mul(aggs_t[:, cs], aggp[0:DIM, :], rb_s)

            # phase 2: sequential t-chain, per node half.
            # prev-state matmuls accumulate incrementally into persistent
            # PSUM banks (w2 / wf block weights are t-invariant), read
            # mid-group by the relus.
            for h in range(2):
                cs = slice(h * NH, (h + 1) * NH)
                p2 = pp2.tile([8, NH], F32, tag="p2")
                pf = ppf.tile([DIM, NH], F32, tag="pf")
                for t in range(T):
                    r8 = slice(t * 8, t * 8 + 8)
                    p1 = pp1.tile([8, NH], F32, tag="p1")
                    nc.tensor.matmul(p1, wraw_t[:, r8], dt_t[:, cs],
                                     start=True, stop=False)
                    nc.tensor.matmul(p1, wagg_t[:, r8], aggs_t[:, cs],
                                     start=False, stop=(t == 0))
                    if t > 0:
                        nc.tensor.matmul(p1, wprev_t[:, r8], prev_t[:, cs],
                                         start=False, stop=True)
                    # h(t) = relu(p1) + pos(t)   (fused on DVE)
                    nc.vector.scalar_tensor_tensor(
                        h2_t[:, t, cs], p1, 0.0, pt_t[:, t, cs],
                        op0=mybir.AluOpType.max, op1=mybir.AluOpType.add)
                    # p2 += w2s[t]^T @ h(t);  prev = relu(p2)
                    nc.tensor.matmul(p2, w2_t[:, r8], h2_t[:, t, cs],
                                     start=(t == 0), stop=(t == T - 1),
                                     skip_group_check=True)
                    if t < T - 1:
                        nc.scalar.activation(prev_t[:, cs], p2,
                                             mybir.ActivationFunctionType.Relu)
                    # pf += wf[t-block]^T @ h(t)
                    nc.tensor.matmul(pf, wf_t[:, t * DIM:(t + 1) * DIM],
                                     h2_t[:, t, cs],
                                     start=(t == 0), stop=(t == T - 1),
                                     skip_group_check=True)
                # final: out = relu(pf)
                nc.scalar.activation(outt_t[:, cs], pf,
                                     mybir.ActivationFunctionType.Relu)
                nc.sync.dma_start(out=out_d[:, cs], in_=outt_t[:, cs])

    split_multi_waits(nc)
    return nc


def prep_in_maps(adj, data, pos, his_W, cur_W, his_weight, cur_weight,
                 final_weight):
    adj = np.asarray(adj, dtype=np.float32)
    data = np.asarray(data, dtype=np.float32)
    pos = np.asarray(pos, dtype=np.float32)
    his_W = np.asarray(his_W, dtype=np.float32)
    cur_W = np.asarray(cur_W, dtype=np.float32)
    his_weight = np.asarray(his_weight, dtype=np.float32)
    cur_weight = np.asarray(cur_weight, dtype=np.float32)
    final_weight = np.asarray(final_weight, dtype=np.float32)

    # X = data rearranged [N, 96] (col = t*8+d), plus ones column
    X = np.ascontiguousarray(data.transpose(1, 0, 2).reshape(N, DIM))
    Xe = np.concatenate([X, np.ones((N, 1), np.float32)], axis=1)
    # pre-tiled for DMA: xe[p, k, c] = Xe[k*KT+p, c]
    xe_h = np.ascontiguousarray(
        Xe.reshape(NKT, KT, XW).transpose(1, 0, 2)).astype(BF16_NP)

    adjT = np.ascontiguousarray(adj.T).astype(BF16_NP)

    # weight packing (zero-padded block maps, see build_nc layout)
    wraw = np.zeros((DIM, DIM), np.float32)
    wagg = np.zeros((DIM, DIM), np.float32)
    wprev = np.zeros((8, DIM), np.float32)
    for t in range(T):
        wraw[t * 8:t * 8 + 7, t * 8:t * 8 + 7] = his_W[t][:, 0:7].T
        wraw[t * 8 + 7, t * 8 + 7] = cur_W[t][0, 0]
        wagg[t * 8:t * 8 + 7, t * 8:t * 8 + 7] = his_W[t][:, 7:14].T
        wagg[t * 8 + 7, t * 8 + 7] = cur_W[t][0, 1]
        wprev[0:7, t * 8:t * 8 + 7] = his_W[t][:, 21:28].T
        wprev[7, t * 8 + 7] = cur_W[t][0, 3]
    # w2s[d, 8t'+o] = prev-update weight from h(t') feature d to output o;
    # t-invariant blocks, accumulated incrementally on-chip.
    w2 = np.zeros((8, DIM), np.float32)
    for tp in range(T):
        w2[0:7, tp * 8:tp * 8 + 7] = his_weight[:, 7 * tp:7 * tp + 7].T
        w2[7, tp * 8 + 7] = cur_weight[0, tp]
    # interleaved feature (8t+d) -> reference feature (7t+d | 84+t)
    f_ref = np.array([7 * t + d if d < 7 else 84 + t
                      for t in range(T) for d in range(8)])
    wf96 = final_weight[:, f_ref].T  # [96 (8t+d), 96 (out)]
    # wf3[d, t*96+o] = wf96[8t+d, o]
    wf = np.ascontiguousarray(
        wf96.reshape(T, 8, DIM).transpose(1, 0, 2).reshape(8, T * DIM))

    in_maps = []
    for c in range(NCORES):
        c0 = c * NPC
        ac = np.zeros((N, NP), BF16_NP)
        ac[:, :NPC] = adjT[:, c0:c0 + NPC]
        # a[h, p, k, n] = ac[k*KT+p, h*NH+n]
        ah = np.ascontiguousarray(
            ac.reshape(NKT, KT, 2, NH).transpose(2, 1, 0, 3))
        dtc = np.zeros((DIM, NP), np.float32)
        dtc[:, :NPC] = data[:, c0:c0 + NPC, :].transpose(0, 2, 1).reshape(
            DIM, NPC)
        ptc = np.zeros((8, T, NP), np.float32)
        ptc[:, :, :NPC] = pos[:, c0:c0 + NPC, :].transpose(2, 0, 1)
        in_maps.append({
            "a": ah, "xe": xe_h, "dt": dtc, "pt": ptc,
            "wraw": wraw, "wagg": wagg, "wprev": wprev, "w2": w2, "wf": wf,
        })
    return in_maps


def assemble(results):
    out = np.empty((N, DIM), np.float32)
    for c in range(NCORES):
        out[c * NPC:(c + 1) * NPC, :] = results[c]["out"][:, :NPC].T
    return out


_NC_CACHE = None


def get_nc():
    global _NC_CACHE
    if _NC_CACHE is None:
        _NC_CACHE = build_nc()
    return _NC_CACHE


def run_spmd(in_maps, **kwargs):
    nc = get_nc()
    return bass_utils.run_bass_kernel_spmd(
        nc, in_maps, list(range(NCORES)), **kwargs)


def kernel(**inputs):
    in_maps = prep_in_maps(**inputs)
    res = run_spmd(in_maps)
    return assemble(res.results)


# revision 18
# speedup vs baseline: 1.5764x; 1.5764x over previous
"""Trainium2 Bass kernel for nn_CombinedGNN (gnn_message_passing).

Strategy (8 NeuronCores, node/row parallel, zero collectives):
  - masks[1] in the reference is identically zero (elementwise pow of a 0/1
    matrix), so only mask0 = adj/rowdeg matters.
  - All T=12 timesteps' aggregations are mask0 @ data[t] -> batched into ONE
    matmul  adj @ [X | 1]  with X = data rearranged to [N, 96]; the ones
    column yields row degrees, and the 1/deg row scaling is applied after.
  - Each core owns 625 nodes (padded to 640). It gets adj^T's column block
    (so the contraction dim sits on SBUF partitions with contiguous DMA) and
    computes its nodes' full output independently.
  - The sequential t-chain (his_prev/cur_prev recurrences) runs in
    [feature-on-partition, node-on-free] orientation with host-prepacked /
    permuted weight matrices so no on-chip transposes are needed.
  - adj (exactly representable 0/1) and X are cast to bf16 for the big
    matmul; accumulation is fp32 in PSUM. Everything downstream is fp32.
"""

import numpy as np
import ml_dtypes

import concourse.bass as bass
import concourse.mybir as mybir
import concourse.bass_utils as bass_utils
from concourse.tile import TileContext

# problem constants (hardcoded per harness contract)
N, T, DAY, L = 5000, 12, 8, 2
F = DAY - 1
DIM = T * DAY  # 96
NCORES = 8
NPC = N // NCORES        # 625 nodes per core
NP = 640                 # padded nodes per core
NH = NP // 2             # 320, node half processed per psum chunk
KT = 125                 # contraction tile (partitions)
NKT = N // KT            # 40
KG = 10                  # k-tiles per DMA group
NG = NKT // KG           # 4
XW = DIM + 1             # 97: 96 features + ones column

F32 = mybir.dt.float32
BF16 = mybir.dt.bfloat16
BF16_NP = ml_dtypes.bfloat16

_MAXW = 1


def split_multi_waits(nc):
    """Walrus in this container rejects instructions with >~2 sync waits.
    Hoist extra waits onto preceding single-wait NoOps on the same engine."""
    f = nc.m.functions[0]
    for bb in list(f.blocks):
        new, ctr = [], 0
        for inst in bb.instructions:
            si = inst.sync_info
            waits = list(si.on_wait) if (si and si.on_wait) else []
            if len(waits) > _MAXW:
                head, keep = waits[:-_MAXW], waits[-_MAXW:]
                for i in range(0, len(head), _MAXW):
                    nop = mybir.InstNoOp(
                        name=f"{inst.name}-wsplit{ctr}", engine=inst.engine,
                        ins=[], outs=[],
                        sync_info=mybir.SyncInfo(on_wait=head[i:i + _MAXW],
                                                 on_update=[]),
                    )
                    ctr += 1
                    new.append(nop)
                inst.sync_info = mybir.SyncInfo(
                    on_wait=keep,
                    on_update=list(si.on_update) if si.on_update else [])
            new.append(inst)
        bb.instructions = new


def build_nc():
    nc = bass.Bass()
    a_d = nc.dram_tensor("a", [2, KT, NKT, NH], BF16, kind="ExternalInput")
    xe_d = nc.dram_tensor("xe", [KT, NKT, XW], BF16, kind="ExternalInput")
    dt_d = nc.dram_tensor("dt", [DIM, NP], BF16, kind="ExternalInput")
    pt_d = nc.dram_tensor("pt", [8, T, NP], F32, kind="ExternalInput")
    wraw_d = nc.dram_tensor("wraw", [DIM, DIM], BF16, kind="ExternalInput")
    wagg_d = nc.dram_tensor("wagg", [DIM, DIM], BF16, kind="ExternalInput")
    wprev_d = nc.dram_tensor("wprev", [8, DIM], BF16, kind="ExternalInput")
    w2_d = nc.dram_tensor("w2", [8, DIM], BF16, kind="ExternalInput")
    wf_d = nc.dram_tensor("wf", [8, T * DIM], BF16, kind="ExternalInput")
    out_d = nc.dram_tensor("out", [DIM, NP], F32, kind="ExternalOutput")

    with TileContext(nc) as tc:
        with (
            tc.tile_pool(name="const", bufs=1) as cpool,
            tc.tile_pool(name="adma", bufs=3) as apool,
            tc.tile_pool(name="work", bufs=2) as wpool,
            tc.tile_pool(name="pagg", bufs=1, space="PSUM") as pagg,
            tc.tile_pool(name="pbc", bufs=1, space="PSUM") as pbc,
            tc.tile_pool(name="pp1", bufs=2, space="PSUM") as pp1,
            tc.tile_pool(name="pp2", bufs=2, space="PSUM") as pp2,
            tc.tile_pool(name="ppf", bufs=2, space="PSUM") as ppf,
        ):
            # a-tile DMAs ride the SP HWDGE ring (critical path, issued
            # first); all constant loads go on the ACT HWDGE ring.
            a_tiles = {}
            for h in range(2):
                for g in range(NG):
                    a_t = apool.tile([KT, KG, NH], BF16, tag="a")
                    nc.sync.dma_start(
                        out=a_t, in_=a_d[h, :, g * KG:(g + 1) * KG, :])
                    a_tiles[(h, g)] = a_t

            # constants / persistent state
            xe_t = cpool.tile([KT, NKT, XW], BF16)
            nc.scalar.dma_start(out=xe_t, in_=xe_d[:, :, :])
            dt_t = cpool.tile([DIM, NP], BF16)
            nc.scalar.dma_start(out=dt_t, in_=dt_d[:, :])
            pt_t = cpool.tile([8, T, NP], F32)
            nc.scalar.dma_start(out=pt_t, in_=pt_d[:, :, :])
            wraw_t = cpool.tile([DIM, DIM], BF16)
            nc.scalar.dma_start(out=wraw_t, in_=wraw_d[:, :])
            wagg_t = cpool.tile([DIM, DIM], BF16)
            nc.scalar.dma_start(out=wagg_t, in_=wagg_d[:, :])
            wprev_t = cpool.tile([8, DIM], BF16)
            nc.scalar.dma_start(out=wprev_t, in_=wprev_d[:, :])
            w2_t = cpool.tile([8, DIM], BF16)
            nc.scalar.dma_start(out=w2_t, in_=w2_d[:, :])
            wf_t = cpool.tile([8, T * DIM], BF16)
            nc.scalar.dma_start(out=wf_t, in_=wf_d[:, :])

            ones_t = cpool.tile([1, DIM], F32)
            nc.vector.memset(ones_t, 1.0)
            h2_t = cpool.tile([8, T, NP], BF16)
            prev_t = cpool.tile([8, NP], BF16)
            nc.vector.memset(prev_t, 0.0)
            aggs_t = cpool.tile([DIM, NP], BF16)
            outt_t = cpool.tile([DIM, NP], F32)

            # phase 1: aggT[97, NH] = [X|1]^T @ adjT_shard  per node half
            for h in range(2):
                cs = slice(h * NH, (h + 1) * NH)
                aggp = pagg.tile([XW, NH], F32, tag="aggp")
                for g in range(NG):
                    a_t = a_tiles[(h, g)]
                    for j in range(KG):
                        k = g * KG + j
                        nc.tensor.matmul(aggp, xe_t[:, k, :], a_t[:, j, :],
                                         start=(k == 0), stop=(k == NKT - 1))
                # row-normalization: recip = 1/max(deg,1), broadcast, scale
                degm = wpool.tile([1, NH], F32, tag="degm")
                nc.vector.tensor_scalar_max(degm, aggp[DIM:DIM + 1, :], 1.0)
                recip = wpool.tile([1, NH], F32, tag="recip")
                nc.vector.reciprocal(recip, degm)
                rb_p = pbc.tile([DIM, NH], F32, tag="rbp")
                nc.tensor.matmul(rb_p, ones_t, recip, start=True, stop=True)
                rb_s = wpool.tile([DIM, NH], F32, tag="rbs")
                nc.vector.tensor_copy(rb_s, rb_p)
                nc.vector.tensor_mul(aggs_t[:, cs], aggp[0:DIM, :], rb_s)

            # phase 2: sequential t-chain, per node half.
            # prev-state matmuls accumulate incrementally into persistent
            # PSUM banks (w2 / wf block weights are t-invariant), read
            # mid-group by the relus.
            for h in range(2):
                cs = slice(h * NH, (h + 1) * NH)
                p2 = pp2.tile([8, NH], F32, tag="p2")
                pf = ppf.tile([DIM, NH], F32, tag="pf")
                for t in range(T):
                    r8 = slice(t * 8, t * 8 + 8)
                    p1 = pp1.tile([8, NH], F32, tag="p1")
                    nc.tensor.matmul(p1, wraw_t[:, r8], dt_t[:, cs],
                                     start=True, stop=False)
                    nc.tensor.matmul(p1, wagg_t[:, r8], aggs_t[:, cs],
                                     start=False, stop=(t == 0))
                    if t > 0:
                        nc.tensor.matmul(p1, wprev_t[:, r8], prev_t[:, cs],
                                         start=False, stop=True)
                    # h(t) = relu(p1) + pos(t)   (fused on DVE)
                    nc.vector.scalar_tensor_tensor(
                        h2_t[:, t, cs], p1, 0.0, pt_t[:, t, cs],
                        op0=mybir.AluOpType.max, op1=mybir.AluOpType.add)
                    # p2 += w2s[t]^T @ h(t);  prev = relu(p2)
                    nc.tensor.matmul(p2, w2_t[:, r8], h2_t[:, t, cs],
                                     start=(t == 0), stop=(t == T - 1),
                                     skip_group_check=True)
                    if t < T - 1:
                        nc.scalar.activation(prev_t[:, cs], p2,
                                             mybir.ActivationFunctionType.Relu)
                    # pf += wf[t-block]^T @ h(t)
                    nc.tensor.matmul(pf, wf_t[:, t * DIM:(t + 1) * DIM],
                                     h2_t[:, t, cs],
                                     start=(t == 0), stop=(t == T - 1),
                                     skip_group_check=True)
                # final: out = relu(pf)
                nc.scalar.activation(outt_t[:, cs], pf,
                                     mybir.ActivationFunctionType.Relu)
                nc.sync.dma_start(out=out_d[:, cs], in_=outt_t[:, cs])

    split_multi_waits(nc)
    return nc


def prep_in_maps(adj, data, pos, his_W, cur_W, his_weight, cur_weight,
                 final_weight):
    adj = np.asarray(adj, dtype=np.float32)
    data = np.asarray(data, dtype=np.float32)
    pos = np.asarray(pos, dtype=np.float32)
    his_W = np.asarray(his_W, dtype=np.float32)
    cur_W = np.asarray(cur_W, dtype=np.float32)
    his_weight = np.asarray(his_weight, dtype=np.float32)
    cur_weight = np.asarray(cur_weight, dtype=np.float32)
    final_weight = np.asarray(final_weight, dtype=np.float32)

    # X = data rearranged [N, 96] (col = t*8+d), plus ones column
    X = np.ascontiguousarray(data.transpose(1, 0, 2).reshape(N, DIM))
    Xe = np.concatenate([X, np.ones((N, 1), np.float32)], axis=1)
    # pre-tiled for DMA: xe[p, k, c] = Xe[k*KT+p, c]
    xe_h = np.ascontiguousarray(
        Xe.reshape(NKT, KT, XW).transpose(1, 0, 2)).astype(BF16_NP)

    adjT = np.ascontiguousarray(adj.T).astype(BF16_NP)

    # weight packing (zero-padded block maps, see build_nc layout)
    wraw = np.zeros((DIM, DIM), np.float32)
    wagg = np.zeros((DIM, DIM), np.float32)
    wprev = np.zeros((8, DIM), np.float32)
    for t in range(T):
        wraw[t * 8:t * 8 + 7, t * 8:t * 8 + 7] = his_W[t][:, 0:7].T
        wraw[t * 8 + 7, t * 8 + 7] = cur_W[t][0, 0]
        wagg[t * 8:t * 8 + 7, t * 8:t * 8 + 7] = his_W[t][:, 7:14].T
        wagg[t * 8 + 7, t * 8 + 7] = cur_W[t][0, 1]
        wprev[0:7, t * 8:t * 8 + 7] = his_W[t][:, 21:28].T
        wprev[7, t * 8 + 7] = cur_W[t][0, 3]
    # w2s[d, 8t'+o] = prev-update weight from h(t') feature d to output o;
    # t-invariant blocks, accumulated incrementally on-chip.
    w2 = np.zeros((8, DIM), np.float32)
    for tp in range(T):
        w2[0:7, tp * 8:tp * 8 + 7] = his_weight[:, 7 * tp:7 * tp + 7].T
        w2[7, tp * 8 + 7] = cur_weight[0, tp]
    # interleaved feature (8t+d) -> reference feature (7t+d | 84+t)
    f_ref = np.array([7 * t + d if d < 7 else 84 + t
                      for t in range(T) for d in range(8)])
    wf96 = final_weight[:, f_ref].T  # [96 (8t+d), 96 (out)]
    # wf3[d, t*96+o] = wf96[8t+d, o]
    wf = np.ascontiguousarray(
        wf96.reshape(T, 8, DIM).transpose(1, 0, 2).reshape(8, T * DIM))

    in_maps = []
    for c in range(NCORES):
        c0 = c * NPC
        ac = np.zeros((N, NP), BF16_NP)
        ac[:, :NPC] = adjT[:, c0:c0 + NPC]
        # a[h, p, k, n] = ac[k*KT+p, h*NH+n]
        ah = np.ascontiguousarray(
            ac.reshape(NKT, KT, 2, NH).transpose(2, 1, 0, 3))
        dtc = np.zeros((DIM, NP), np.float32)
        dtc[:, :NPC] = data[:, c0:c0 + NPC, :].transpose(0, 2, 1).reshape(
            DIM, NPC)
        ptc = np.zeros((8, T, NP), np.float32)
        ptc[:, :, :NPC] = pos[:, c0:c0 + NPC, :].transpose(2, 0, 1)
        in_maps.append({
            "a": ah, "xe": xe_h, "dt": dtc.astype(BF16_NP), "pt": ptc,
            "wraw": wraw.astype(BF16_NP), "wagg": wagg.astype(BF16_NP),
            "wprev": wprev.astype(BF16_NP), "w2": w2.astype(BF16_NP),
            "wf": wf.astype(BF16_NP),
        })
    return in_maps


def assemble(results):
    out = np.empty((N, DIM), np.float32)
    for c in range(NCORES):
        out[c * NPC:(c + 1) * NPC, :] = results[c]["out"][:, :NPC].T
    return out


_NC_CACHE = None


def get_nc():
    global _NC_CACHE
    if _NC_CACHE is None:
        _NC_CACHE = build_nc()
    return _NC_CACHE


def run_spmd(in_maps, **kwargs):
    nc = get_nc()
    return bass_utils.run_bass_kernel_spmd(
        nc, in_maps, list(range(NCORES)), **kwargs)


def kernel(**inputs):
    in_maps = prep_in_maps(**inputs)
    res = run_spmd(in_maps)
    return assemble(res.results)


# revision 26
# speedup vs baseline: 1.8820x; 1.1938x over previous
"""Trainium2 Bass kernel for nn_CombinedGNN (gnn_message_passing).

Strategy (8 NeuronCores, node/row parallel, zero collectives):
  - masks[1] in the reference is identically zero (elementwise pow of a 0/1
    matrix), so only mask0 = adj/rowdeg matters.
  - All T=12 timesteps' aggregations are mask0 @ data[t] -> batched into ONE
    matmul  adj @ [X | 1]  with X = data rearranged to [N, 96]; the ones
    column yields row degrees, and the 1/deg row scaling is applied after.
  - Each core owns 625 nodes (padded to 640). It gets adj^T's column block
    (so the contraction dim sits on SBUF partitions with contiguous DMA) and
    computes its nodes' full output independently.
  - The sequential t-chain (his_prev/cur_prev recurrences) runs in
    [feature-on-partition, node-on-free] orientation with host-prepacked /
    permuted weight matrices so no on-chip transposes are needed.
  - adj (exactly representable 0/1) and X are cast to bf16 for the big
    matmul; accumulation is fp32 in PSUM. Everything downstream is fp32.
"""

import numpy as np
import ml_dtypes

import concourse.bass as bass
import concourse.mybir as mybir
import concourse.bass_utils as bass_utils
from concourse.tile import TileContext

# problem constants (hardcoded per harness contract)
N, T, DAY, L = 5000, 12, 8, 2
F = DAY - 1
DIM = T * DAY  # 96
NCORES = 8
NPC = N // NCORES        # 625 nodes per core
NP = 640                 # padded nodes per core
NH = NP // 2             # 320, node half processed per psum chunk
KT = 125                 # contraction tile (partitions)
NKT = N // KT            # 40
KG = 20                  # k-tiles per DMA group
NG = NKT // KG           # 2
XW = DIM + 1             # 97: 96 features + ones column

F32 = mybir.dt.float32
BF16 = mybir.dt.bfloat16
BF16_NP = ml_dtypes.bfloat16

_MAXW = 1


def split_multi_waits(nc):
    """Walrus in this container rejects instructions with >~2 sync waits.
    Hoist extra waits onto preceding single-wait NoOps on the same engine."""
    f = nc.m.functions[0]
    for bb in list(f.blocks):
        new, ctr = [], 0
        for inst in bb.instructions:
            si = inst.sync_info
            waits = list(si.on_wait) if (si and si.on_wait) else []
            if len(waits) > _MAXW:
                head, keep = waits[:-_MAXW], waits[-_MAXW:]
                for i in range(0, len(head), _MAXW):
                    nop = mybir.InstNoOp(
                        name=f"{inst.name}-wsplit{ctr}", engine=inst.engine,
                        ins=[], outs=[],
                        sync_info=mybir.SyncInfo(on_wait=head[i:i + _MAXW],
                                                 on_update=[]),
                    )
                    ctr += 1
                    new.append(nop)
                inst.sync_info = mybir.SyncInfo(
                    on_wait=keep,
                    on_update=list(si.on_update) if si.on_update else [])
            new.append(inst)
        bb.instructions = new


def build_nc():
    nc = bass.Bass()
    a_d = nc.dram_tensor("a", [2, KT, NKT, NH], BF16, kind="ExternalInput")
    xe_d = nc.dram_tensor("xe", [KT, NKT, XW], BF16, kind="ExternalInput")
    dt_d = nc.dram_tensor("dt", [DIM, NP], BF16, kind="ExternalInput")
    pt_d = nc.dram_tensor("pt", [8, T, NP], F32, kind="ExternalInput")
    # w_rp: [104, 96] — rows 0:96 raw block-diag, rows 96:104 prev block
    wrp_d = nc.dram_tensor("wrp", [104, DIM], BF16, kind="ExternalInput")
    wagg_d = nc.dram_tensor("wagg", [DIM, DIM], BF16, kind="ExternalInput")
    # wcomb: [8, T, 104] — cols 0:96 wf block(t), cols 96:104 w2s block(t)
    wcomb_d = nc.dram_tensor("wcomb", [8, T * 104], BF16,
                             kind="ExternalInput")
    out_d = nc.dram_tensor("out", [DIM, NP], F32, kind="ExternalOutput")

    with TileContext(nc) as tc:
        with (
            tc.tile_pool(name="const", bufs=1) as cpool,
            tc.tile_pool(name="adma", bufs=3) as apool,
            tc.tile_pool(name="work", bufs=2) as wpool,
            tc.tile_pool(name="pagg", bufs=1, space="PSUM") as pagg,
            tc.tile_pool(name="pbc", bufs=1, space="PSUM") as pbc,
            tc.tile_pool(name="pp1", bufs=3, space="PSUM") as pp1,
            tc.tile_pool(name="pcm", bufs=2, space="PSUM") as pcm,
        ):
            # a-tile DMAs ride the SP HWDGE ring (critical path, issued
            # first); all constant loads go on the ACT HWDGE ring.
            a_tiles = {}
            for h in range(2):
                for g in range(NG):
                    a_t = apool.tile([KT, KG, NH], BF16, tag="a")
                    nc.sync.dma_start(
                        out=a_t, in_=a_d[h, :, g * KG:(g + 1) * KG, :])
                    a_tiles[(h, g)] = a_t

            # constants / persistent state
            xe_t = cpool.tile([KT, NKT, XW], BF16)
            nc.scalar.dma_start(out=xe_t, in_=xe_d[:, :, :])
            # dtprev: rows 0:96 = dataT (static), rows 96:104 = prev state
            dtprev_t = cpool.tile([104, NP], BF16)
            nc.scalar.dma_start(out=dtprev_t[0:DIM, :], in_=dt_d[:, :])
            nc.vector.memset(dtprev_t[DIM:104, :], 0.0)
            pt_t = cpool.tile([8, T, NP], F32)
            nc.scalar.dma_start(out=pt_t, in_=pt_d[:, :, :])
            wrp_t = cpool.tile([104, DIM], BF16)
            nc.scalar.dma_start(out=wrp_t, in_=wrp_d[:, :])
            wagg_t = cpool.tile([DIM, DIM], BF16)
            nc.scalar.dma_start(out=wagg_t, in_=wagg_d[:, :])
            wcomb_t = cpool.tile([8, T * 104], BF16)
            nc.scalar.dma_start(out=wcomb_t, in_=wcomb_d[:, :])

            ones_t = cpool.tile([1, DIM], F32)
            nc.vector.memset(ones_t, 1.0)
            h2_t = cpool.tile([8, T, NP], BF16)
            aggs_t = cpool.tile([DIM, NP], BF16)
            outt_t = cpool.tile([DIM, NP], F32)

            # phase 1: aggT[97, NH] = [X|1]^T @ adjT_shard  per node half
            for h in range(2):
                cs = slice(h * NH, (h + 1) * NH)
                aggp = pagg.tile([XW, NH], F32, tag="aggp")
                for g in range(NG):
                    a_t = a_tiles[(h, g)]
                    for j in range(KG):
                        k = g * KG + j
                        nc.tensor.matmul(aggp, xe_t[:, k, :], a_t[:, j, :],
                                         start=(k == 0), stop=(k == NKT - 1))
                # row-normalization: recip = 1/max(deg,1), broadcast, scale
                degm = wpool.tile([1, NH], F32, tag="degm")
                nc.vector.tensor_scalar_max(degm, aggp[DIM:DIM + 1, :], 1.0)
                recip = wpool.tile([1, NH], F32, tag="recip")
                nc.vector.reciprocal(recip, degm)
                rb_p = pbc.tile([DIM, NH], F32, tag="rbp")
                nc.tensor.matmul(rb_p, ones_t, recip, start=True, stop=True)
                rb_s = wpool.tile([DIM, NH], F32, tag="rbs")
                nc.vector.tensor_copy(rb_s, rb_p)
                nc.vector.tensor_mul(aggs_t[:, cs], aggp[0:DIM, :], rb_s)

            # phase 2: sequential t-chain, both node halves interleaved.
            # Per t and half: 3 matmuls —
            #   p1 = w_rp[t]^T @ [dt; prev]  (chain) + wagg[t]^T @ aggs
            #   pcomb += wcomb[t]^T @ h(t)   (rows 0:96 = final acc,
            #                                 rows 96:104 = prev-update acc)
            pcombs = [pcm.tile([104, NH], F32, tag="pcm", name=f"pcomb{h}")
                      for h in range(2)]
            for t in range(T):
                r8 = slice(t * 8, t * 8 + 8)
                for h in range(2):
                    cs = slice(h * NH, (h + 1) * NH)
                    pcomb = pcombs[h]
                    p1 = pp1.tile([8, NH], F32, tag="p1")
                    nc.tensor.matmul(p1, wagg_t[:, r8], aggs_t[:, cs],
                                     start=True, stop=False)
                    nc.tensor.matmul(p1, wrp_t[:, r8], dtprev_t[:, cs],
                                     start=False, stop=True)
                    # h(t) = relu(p1) + pos(t)   (fused on DVE)
                    nc.vector.scalar_tensor_tensor(
                        h2_t[:, t, cs], p1, 0.0, pt_t[:, t, cs],
                        op0=mybir.AluOpType.max, op1=mybir.AluOpType.add)
                    # pcomb += wcomb[t]^T @ h(t)
                    nc.tensor.matmul(pcomb,
                                     wcomb_t[:, t * 104:(t + 1) * 104],
                                     h2_t[:, t, cs],
                                     start=(t == 0), stop=(t == T - 1),
                                     skip_group_check=True)
                    # prev = relu(p2 rows)  (DVE)
                    if t < T - 1:
                        nc.vector.tensor_scalar_max(
                            dtprev_t[DIM:104, cs], pcomb[DIM:104, :], 0.0)
            # final: out = relu(pcomb rows 0:96)
            for h in range(2):
                cs = slice(h * NH, (h + 1) * NH)
                nc.vector.tensor_scalar_max(outt_t[:, cs],
                                            pcombs[h][0:DIM, :], 0.0)
                nc.sync.dma_start(out=out_d[:, cs], in_=outt_t[:, cs])

    split_multi_waits(nc)
    return nc


def prep_in_maps(adj, data, pos, his_W, cur_W, his_weight, cur_weight,
                 final_weight):
    adj = np.asarray(adj, dtype=np.float32)
    data = np.asarray(data, dtype=np.float32)
    pos = np.asarray(pos, dtype=np.float32)
    his_W = np.asarray(his_W, dtype=np.float32)
    cur_W = np.asarray(cur_W, dtype=np.float32)
    his_weight = np.asarray(his_weight, dtype=np.float32)
    cur_weight = np.asarray(cur_weight, dtype=np.float32)
    final_weight = np.asarray(final_weight, dtype=np.float32)

    # X = data rearranged [N, 96] (col = t*8+d), plus ones column
    X = np.ascontiguousarray(data.transpose(1, 0, 2).reshape(N, DIM))
    Xe = np.concatenate([X, np.ones((N, 1), np.float32)], axis=1)
    # pre-tiled for DMA: xe[p, k, c] = Xe[k*KT+p, c]
    xe_h = np.ascontiguousarray(
        Xe.reshape(NKT, KT, XW).transpose(1, 0, 2)).astype(BF16_NP)

    adjT = np.ascontiguousarray(adj.T).astype(BF16_NP)

    # weight packing (zero-padded block maps, see build_nc layout)
    wraw = np.zeros((DIM, DIM), np.float32)
    wagg = np.zeros((DIM, DIM), np.float32)
    wprev = np.zeros((8, DIM), np.float32)
    for t in range(T):
        wraw[t * 8:t * 8 + 7, t * 8:t * 8 + 7] = his_W[t][:, 0:7].T
        wraw[t * 8 + 7, t * 8 + 7] = cur_W[t][0, 0]
        wagg[t * 8:t * 8 + 7, t * 8:t * 8 + 7] = his_W[t][:, 7:14].T
        wagg[t * 8 + 7, t * 8 + 7] = cur_W[t][0, 1]
        wprev[0:7, t * 8:t * 8 + 7] = his_W[t][:, 21:28].T
        wprev[7, t * 8 + 7] = cur_W[t][0, 3]
    # w2s[d, 8t'+o] = prev-update weight from h(t') feature d to output o;
    # t-invariant blocks, accumulated incrementally on-chip.
    w2 = np.zeros((8, DIM), np.float32)
    for tp in range(T):
        w2[0:7, tp * 8:tp * 8 + 7] = his_weight[:, 7 * tp:7 * tp + 7].T
        w2[7, tp * 8 + 7] = cur_weight[0, tp]
    # interleaved feature (8t+d) -> reference feature (7t+d | 84+t)
    f_ref = np.array([7 * t + d if d < 7 else 84 + t
                      for t in range(T) for d in range(8)])
    wf96 = final_weight[:, f_ref].T  # [96 (8t+d), 96 (out)]
    # wf3[d, t*96+o] = wf96[8t+d, o]
    wf = np.ascontiguousarray(
        wf96.reshape(T, 8, DIM).transpose(1, 0, 2).reshape(8, T * DIM))
    # merged lhsT blocks:
    # wrp [104, 96]: rows 0:96 = wraw block-diag, rows 96:104 = wprev
    wrp = np.concatenate([wraw, wprev], axis=0)
    # wcomb [8, T*104]: per t, cols 0:96 = wf block(t), cols 96:104 = w2s(t)
    wcomb = np.zeros((8, T, 104), np.float32)
    for t in range(T):
        wcomb[:, t, 0:DIM] = wf[:, t * DIM:(t + 1) * DIM]
        wcomb[:, t, DIM:104] = w2[:, t * 8:(t + 1) * 8]
    wcomb = np.ascontiguousarray(wcomb.reshape(8, T * 104))

    in_maps = []
    for c in range(NCORES):
        c0 = c * NPC
        ac = np.zeros((N, NP), BF16_NP)
        ac[:, :NPC] = adjT[:, c0:c0 + NPC]
        # a[h, p, k, n] = ac[k*KT+p, h*NH+n]
        ah = np.ascontiguousarray(
            ac.reshape(NKT, KT, 2, NH).transpose(2, 1, 0, 3))
        dtc = np.zeros((DIM, NP), np.float32)
        dtc[:, :NPC] = data[:, c0:c0 + NPC, :].transpose(0, 2, 1).reshape(
            DIM, NPC)
        ptc = np.zeros((8, T, NP), np.float32)
        ptc[:, :, :NPC] = pos[:, c0:c0 + NPC, :].transpose(2, 0, 1)
        in_maps.append({
            "a": ah, "xe": xe_h, "dt": dtc.astype(BF16_NP), "pt": ptc,
            "wrp": wrp.astype(BF16_NP), "wagg": wagg.astype(BF16_NP),
            "wcomb": wcomb.astype(BF16_NP),
        })
    return in_maps


def assemble(results):
    out = np.empty((N, DIM), np.float32)
    for c in range(NCORES):
        out[c * NPC:(c + 1) * NPC, :] = results[c]["out"][:, :NPC].T
    return out


_NC_CACHE = None


def get_nc():
    global _NC_CACHE
    if _NC_CACHE is None:
        _NC_CACHE = build_nc()
    return _NC_CACHE


def run_spmd(in_maps, **kwargs):
    nc = get_nc()
    return bass_utils.run_bass_kernel_spmd(
        nc, in_maps, list(range(NCORES)), **kwargs)


def kernel(**inputs):
    in_maps = prep_in_maps(**inputs)
    res = run_spmd(in_maps)
    return assemble(res.results)


# revision 29
# speedup vs baseline: 2.3812x; 1.2652x over previous
"""Trainium2 Bass kernel for nn_CombinedGNN (gnn_message_passing).

Strategy (8 NeuronCores, node/row parallel, zero collectives):
  - masks[1] in the reference is identically zero (elementwise pow of a 0/1
    matrix), so only mask0 = adj/rowdeg matters.
  - All T=12 timesteps' aggregations are mask0 @ data[t] -> batched into ONE
    matmul  adj @ [X | 1]  with X = data rearranged to [N, 96]; the ones
    column yields row degrees, and the 1/deg row scaling is applied after.
  - Each core owns 625 nodes (padded to 640). It gets adj^T's column block
    (so the contraction dim sits on SBUF partitions with contiguous DMA) and
    computes its nodes' full output independently.
  - The sequential t-chain (his_prev/cur_prev recurrences) runs in
    [feature-on-partition, node-on-free] orientation with host-prepacked /
    permuted weight matrices so no on-chip transposes are needed.
  - adj (exactly representable 0/1) and X are cast to bf16 for the big
    matmul; accumulation is fp32 in PSUM. Everything downstream is fp32.
"""

import numpy as np
import ml_dtypes

import concourse.bass as bass
import concourse.mybir as mybir
import concourse.bass_utils as bass_utils
from concourse.tile import TileContext

# problem constants (hardcoded per harness contract)
N, T, DAY, L = 5000, 12, 8, 2
F = DAY - 1
DIM = T * DAY  # 96
NCORES = 8
NPC = N // NCORES        # 625 nodes per core
NP = 640                 # padded nodes per core
NH = NP // 2             # 320, node half processed per psum chunk
KT = 128                 # contraction tile (partitions; K padded to 5120)
NK = 5120                # padded contraction size
NKT = NK // KT           # 40
KG = 20                  # k-tiles per DMA group
NG = NKT // KG           # 2
XW = DIM + 1             # 97: 96 features + ones column

F32 = mybir.dt.float32
BF16 = mybir.dt.bfloat16
BF16_NP = ml_dtypes.bfloat16

_MAXW = 1


def split_multi_waits(nc):
    """Walrus in this container rejects instructions with >~2 sync waits.
    Hoist extra waits onto preceding single-wait NoOps on the same engine."""
    f = nc.m.functions[0]
    for bb in list(f.blocks):
        new, ctr = [], 0
        for inst in bb.instructions:
            si = inst.sync_info
            waits = list(si.on_wait) if (si and si.on_wait) else []
            if len(waits) > _MAXW:
                head, keep = waits[:-_MAXW], waits[-_MAXW:]
                for i in range(0, len(head), _MAXW):
                    nop = mybir.InstNoOp(
                        name=f"{inst.name}-wsplit{ctr}", engine=inst.engine,
                        ins=[], outs=[],
                        sync_info=mybir.SyncInfo(on_wait=head[i:i + _MAXW],
                                                 on_update=[]),
                    )
                    ctr += 1
                    new.append(nop)
                inst.sync_info = mybir.SyncInfo(
                    on_wait=keep,
                    on_update=list(si.on_update) if si.on_update else [])
            new.append(inst)
        bb.instructions = new


def build_nc():
    nc = bass.Bass()
    a_d = nc.dram_tensor("a", [2, KT, NKT, NH], BF16, kind="ExternalInput")
    xe_d = nc.dram_tensor("xe", [KT, NKT, XW], BF16, kind="ExternalInput")
    dt_d = nc.dram_tensor("dt", [DIM, NP], BF16, kind="ExternalInput")
    pt_d = nc.dram_tensor("pt", [8, T, NP], F32, kind="ExternalInput")
    # w_rp: [104, 96] — rows 0:96 raw block-diag, rows 96:104 prev block
    wrp_d = nc.dram_tensor("wrp", [104, DIM], BF16, kind="ExternalInput")
    wagg_d = nc.dram_tensor("wagg", [DIM, DIM], BF16, kind="ExternalInput")
    # wcomb: [8, T, 104] — cols 0:96 wf block(t), cols 96:104 w2s block(t)
    wcomb_d = nc.dram_tensor("wcomb", [8, T * 104], BF16,
                             kind="ExternalInput")
    out_d = nc.dram_tensor("out", [DIM, NP], F32, kind="ExternalOutput")

    with TileContext(nc) as tc:
        with (
            tc.tile_pool(name="const", bufs=1) as cpool,
            tc.tile_pool(name="adma", bufs=3) as apool,
            tc.tile_pool(name="work", bufs=2) as wpool,
            tc.tile_pool(name="pagg", bufs=1, space="PSUM") as pagg,
            tc.tile_pool(name="pbc", bufs=1, space="PSUM") as pbc,
            tc.tile_pool(name="pp1", bufs=3, space="PSUM") as pp1,
            tc.tile_pool(name="pcm", bufs=2, space="PSUM") as pcm,
        ):
            # a-tile DMAs ride the SP HWDGE ring (critical path, issued
            # first); all constant loads go on the ACT HWDGE ring.
            a_tiles = {}
            for h in range(2):
                for g in range(NG):
                    a_t = apool.tile([KT, KG, NH], BF16, tag="a")
                    nc.sync.dma_start(
                        out=a_t, in_=a_d[h, :, g * KG:(g + 1) * KG, :])
                    a_tiles[(h, g)] = a_t

            # constants / persistent state
            xe_t = cpool.tile([KT, NKT, XW], BF16)
            nc.scalar.dma_start(out=xe_t, in_=xe_d[:, :, :])
            # dtprev: rows 0:96 = dataT (static), rows 96:104 = prev state
            dtprev_t = cpool.tile([104, NP], BF16)
            nc.scalar.dma_start(out=dtprev_t[0:DIM, :], in_=dt_d[:, :])
            nc.vector.memset(dtprev_t[DIM:104, :], 0.0)
            pt_t = cpool.tile([8, T, NP], F32)
            nc.scalar.dma_start(out=pt_t, in_=pt_d[:, :, :])
            wrp_t = cpool.tile([104, DIM], BF16)
            nc.scalar.dma_start(out=wrp_t, in_=wrp_d[:, :])
            wagg_t = cpool.tile([DIM, DIM], BF16)
            nc.scalar.dma_start(out=wagg_t, in_=wagg_d[:, :])
            wcomb_t = cpool.tile([8, T * 104], BF16)
            nc.scalar.dma_start(out=wcomb_t, in_=wcomb_d[:, :])

            ones_t = cpool.tile([1, DIM], F32)
            nc.vector.memset(ones_t, 1.0)
            h2_t = cpool.tile([8, T, NP], BF16)
            aggs_t = cpool.tile([DIM, NP], BF16)
            outt_t = cpool.tile([DIM, NP], F32)

            # phase 1: aggT[97, NH] = [X|1]^T @ adjT_shard  per node half
            for h in range(2):
                cs = slice(h * NH, (h + 1) * NH)
                aggp = pagg.tile([XW, NH], F32, tag="aggp")
                for g in range(NG):
                    a_t = a_tiles[(h, g)]
                    for j in range(KG):
                        k = g * KG + j
                        nc.tensor.matmul(aggp, xe_t[:, k, :], a_t[:, j, :],
                                         start=(k == 0), stop=(k == NKT - 1))
                # row-normalization: recip = 1/max(deg,1), broadcast, scale
                degm = wpool.tile([1, NH], F32, tag="degm")
                nc.vector.tensor_scalar_max(degm, aggp[DIM:DIM + 1, :], 1.0)
                recip = wpool.tile([1, NH], F32, tag="recip")
                nc.vector.reciprocal(recip, degm)
                rb_p = pbc.tile([DIM, NH], F32, tag="rbp")
                nc.tensor.matmul(rb_p, ones_t, recip, start=True, stop=True)
                rb_s = wpool.tile([DIM, NH], F32, tag="rbs")
                nc.vector.tensor_copy(rb_s, rb_p)
                nc.vector.tensor_mul(aggs_t[:, cs], aggp[0:DIM, :], rb_s)

            # phase 2: sequential t-chain, both node halves interleaved.
            # Per t and half: 3 matmuls —
            #   p1 = w_rp[t]^T @ [dt; prev]  (chain) + wagg[t]^T @ aggs
            #   pcomb += wcomb[t]^T @ h(t)   (rows 0:96 = final acc,
            #                                 rows 96:104 = prev-update acc)
            pcombs = [pcm.tile([104, NH], F32, tag="pcm", name=f"pcomb{h}")
                      for h in range(2)]
            for t in range(T):
                r8 = slice(t * 8, t * 8 + 8)
                for h in range(2):
                    cs = slice(h * NH, (h + 1) * NH)
                    pcomb = pcombs[h]
                    p1 = pp1.tile([8, NH], F32, tag="p1")
                    nc.tensor.matmul(p1, wagg_t[:, r8], aggs_t[:, cs],
                                     start=True, stop=False)
                    nc.tensor.matmul(p1, wrp_t[:, r8], dtprev_t[:, cs],
                                     start=False, stop=True)
                    # h(t) = relu(p1) + pos(t)   (fused on DVE)
                    nc.vector.scalar_tensor_tensor(
                        h2_t[:, t, cs], p1, 0.0, pt_t[:, t, cs],
                        op0=mybir.AluOpType.max, op1=mybir.AluOpType.add)
                    # pcomb += wcomb[t]^T @ h(t)
                    nc.tensor.matmul(pcomb,
                                     wcomb_t[:, t * 104:(t + 1) * 104],
                                     h2_t[:, t, cs],
                                     start=(t == 0), stop=(t == T - 1),
                                     skip_group_check=True)
                    # prev = relu(p2 rows)  (DVE)
                    if t < T - 1:
                        nc.vector.tensor_scalar_max(
                            dtprev_t[DIM:104, cs], pcomb[DIM:104, :], 0.0)
            # final: out = relu(pcomb rows 0:96)
            for h in range(2):
                cs = slice(h * NH, (h + 1) * NH)
                nc.vector.tensor_scalar_max(outt_t[:, cs],
                                            pcombs[h][0:DIM, :], 0.0)
                nc.sync.dma_start(out=out_d[:, cs], in_=outt_t[:, cs])

    split_multi_waits(nc)
    return nc


def prep_in_maps(adj, data, pos, his_W, cur_W, his_weight, cur_weight,
                 final_weight):
    adj = np.asarray(adj, dtype=np.float32)
    data = np.asarray(data, dtype=np.float32)
    pos = np.asarray(pos, dtype=np.float32)
    his_W = np.asarray(his_W, dtype=np.float32)
    cur_W = np.asarray(cur_W, dtype=np.float32)
    his_weight = np.asarray(his_weight, dtype=np.float32)
    cur_weight = np.asarray(cur_weight, dtype=np.float32)
    final_weight = np.asarray(final_weight, dtype=np.float32)

    # X = data rearranged [N, 96] (col = t*8+d), plus ones column;
    # contraction dim zero-padded to NK=5120 for full-128-partition tiles
    X = np.ascontiguousarray(data.transpose(1, 0, 2).reshape(N, DIM))
    Xe = np.zeros((NK, XW), np.float32)
    Xe[:N, :DIM] = X
    Xe[:N, DIM] = 1.0
    # pre-tiled for DMA: xe[p, k, c] = Xe[k*KT+p, c]
    xe_h = np.ascontiguousarray(
        Xe.reshape(NKT, KT, XW).transpose(1, 0, 2)).astype(BF16_NP)

    adjT = np.ascontiguousarray(adj.T).astype(BF16_NP)

    # weight packing (zero-padded block maps, see build_nc layout)
    wraw = np.zeros((DIM, DIM), np.float32)
    wagg = np.zeros((DIM, DIM), np.float32)
    wprev = np.zeros((8, DIM), np.float32)
    for t in range(T):
        wraw[t * 8:t * 8 + 7, t * 8:t * 8 + 7] = his_W[t][:, 0:7].T
        wraw[t * 8 + 7, t * 8 + 7] = cur_W[t][0, 0]
        wagg[t * 8:t * 8 + 7, t * 8:t * 8 + 7] = his_W[t][:, 7:14].T
        wagg[t * 8 + 7, t * 8 + 7] = cur_W[t][0, 1]
        wprev[0:7, t * 8:t * 8 + 7] = his_W[t][:, 21:28].T
        wprev[7, t * 8 + 7] = cur_W[t][0, 3]
    # w2s[d, 8t'+o] = prev-update weight from h(t') feature d to output o;
    # t-invariant blocks, accumulated incrementally on-chip.
    w2 = np.zeros((8, DIM), np.float32)
    for tp in range(T):
        w2[0:7, tp * 8:tp * 8 + 7] = his_weight[:, 7 * tp:7 * tp + 7].T
        w2[7, tp * 8 + 7] = cur_weight[0, tp]
    # interleaved feature (8t+d) -> reference feature (7t+d | 84+t)
    f_ref = np.array([7 * t + d if d < 7 else 84 + t
                      for t in range(T) for d in range(8)])
    wf96 = final_weight[:, f_ref].T  # [96 (8t+d), 96 (out)]
    # wf3[d, t*96+o] = wf96[8t+d, o]
    wf = np.ascontiguousarray(
        wf96.reshape(T, 8, DIM).transpose(1, 0, 2).reshape(8, T * DIM))
    # merged lhsT blocks:
    # wrp [104, 96]: rows 0:96 = wraw block-diag, rows 96:104 = wprev
    wrp = np.concatenate([wraw, wprev], axis=0)
    # wcomb [8, T*104]: per t, cols 0:96 = wf block(t), cols 96:104 = w2s(t)
    wcomb = np.zeros((8, T, 104), np.float32)
    for t in range(T):
        wcomb[:, t, 0:DIM] = wf[:, t * DIM:(t + 1) * DIM]
        wcomb[:, t, DIM:104] = w2[:, t * 8:(t + 1) * 8]
    wcomb = np.ascontiguousarray(wcomb.reshape(8, T * 104))

    in_maps = []
    for c in range(NCORES):
        c0 = c * NPC
        ac = np.zeros((NK, NP), BF16_NP)
        ac[:N, :NPC] = adjT[:, c0:c0 + NPC]
        # a[h, p, k, n] = ac[k*KT+p, h*NH+n]
        ah = np.ascontiguousarray(
            ac.reshape(NKT, KT, 2, NH).transpose(2, 1, 0, 3))
        dtc = np.zeros((DIM, NP), np.float32)
        dtc[:, :NPC] = data[:, c0:c0 + NPC, :].transpose(0, 2, 1).reshape(
            DIM, NPC)
        ptc = np.zeros((8, T, NP), np.float32)
        ptc[:, :, :NPC] = pos[:, c0:c0 + NPC, :].transpose(2, 0, 1)
        in_maps.append({
            "a": ah, "xe": xe_h, "dt": dtc.astype(BF16_NP), "pt": ptc,
            "wrp": wrp.astype(BF16_NP), "wagg": wagg.astype(BF16_NP),
            "wcomb": wcomb.astype(BF16_NP),
        })
    return in_maps


def assemble(results):
    out = np.empty((N, DIM), np.float32)
    for c in range(NCORES):
        out[c * NPC:(c + 1) * NPC, :] = results[c]["out"][:, :NPC].T
    return out


_NC_CACHE = None


def get_nc():
    global _NC_CACHE
    if _NC_CACHE is None:
        _NC_CACHE = build_nc()
    return _NC_CACHE


def run_spmd(in_maps, **kwargs):
    nc = get_nc()
    return bass_utils.run_bass_kernel_spmd(
        nc, in_maps, list(range(NCORES)), **kwargs)


def kernel(**inputs):
    in_maps = prep_in_maps(**inputs)
    res = run_spmd(in_maps)
    return assemble(res.results)


# revision 32
# speedup vs baseline: 2.7340x; 1.1482x over previous
"""Trainium2 Bass kernel for nn_CombinedGNN (gnn_message_passing).

Strategy (8 NeuronCores, node/row parallel, zero collectives):
  - masks[1] in the reference is identically zero (elementwise pow of a 0/1
    matrix), so only mask0 = adj/rowdeg matters.
  - All T=12 timesteps' aggregations are mask0 @ data[t] -> batched into ONE
    matmul  adj @ [X | 1]  with X = data rearranged to [N, 96]; the ones
    column yields row degrees, and the 1/deg row scaling is applied after.
  - Each core owns 625 nodes (padded to 640). It gets adj^T's column block
    (so the contraction dim sits on SBUF partitions with contiguous DMA) and
    computes its nodes' full output independently.
  - The sequential t-chain (his_prev/cur_prev recurrences) runs in
    [feature-on-partition, node-on-free] orientation with host-prepacked /
    permuted weight matrices so no on-chip transposes are needed.
  - adj (exactly representable 0/1) and X are cast to bf16 for the big
    matmul; accumulation is fp32 in PSUM. Everything downstream is fp32.
"""

import numpy as np
import ml_dtypes

import concourse.bass as bass
import concourse.mybir as mybir
import concourse.bass_utils as bass_utils
from concourse.tile import TileContext

# problem constants (hardcoded per harness contract)
N, T, DAY, L = 5000, 12, 8, 2
F = DAY - 1
DIM = T * DAY  # 96
NCORES = 8
NPC = N // NCORES        # 625 nodes per core
NP = 640                 # padded nodes per core
NH = NP // 2             # 320, node half processed per psum chunk
KT = 128                 # contraction tile (partitions; K padded to 5120)
NK = 5120                # padded contraction size
NKT = NK // KT           # 40
KG = 20                  # k-tiles per DMA group
NG = NKT // KG           # 2
XW = DIM + 1             # 97: 96 features + ones column

F32 = mybir.dt.float32
BF16 = mybir.dt.bfloat16
BF16_NP = ml_dtypes.bfloat16

_MAXW = 1


def split_multi_waits(nc):
    """Walrus in this container rejects instructions with >~2 sync waits.
    Hoist extra waits onto preceding single-wait NoOps on the same engine."""
    f = nc.m.functions[0]
    for bb in list(f.blocks):
        new, ctr = [], 0
        for inst in bb.instructions:
            si = inst.sync_info
            waits = list(si.on_wait) if (si and si.on_wait) else []
            if len(waits) > _MAXW:
                head, keep = waits[:-_MAXW], waits[-_MAXW:]
                for i in range(0, len(head), _MAXW):
                    nop = mybir.InstNoOp(
                        name=f"{inst.name}-wsplit{ctr}", engine=inst.engine,
                        ins=[], outs=[],
                        sync_info=mybir.SyncInfo(on_wait=head[i:i + _MAXW],
                                                 on_update=[]),
                    )
                    ctr += 1
                    new.append(nop)
                inst.sync_info = mybir.SyncInfo(
                    on_wait=keep,
                    on_update=list(si.on_update) if si.on_update else [])
            new.append(inst)
        bb.instructions = new


def build_nc():
    nc = bass.Bass()
    a_d = nc.dram_tensor("a", [2, KT, NKT, NH], BF16, kind="ExternalInput")
    xe_d = nc.dram_tensor("xe", [KT, NKT, XW], BF16, kind="ExternalInput")
    dt_d = nc.dram_tensor("dt", [DIM, NP], BF16, kind="ExternalInput")
    pt_d = nc.dram_tensor("pt", [8, T, NP], F32, kind="ExternalInput")
    # w_rp: [104, 96] — rows 0:96 raw block-diag, rows 96:104 prev block
    wrp_d = nc.dram_tensor("wrp", [104, DIM], BF16, kind="ExternalInput")
    wagg_d = nc.dram_tensor("wagg", [DIM, DIM], BF16, kind="ExternalInput")
    # wcomb: [8, T, 104] — cols 0:96 wf block(t), cols 96:104 w2s block(t)
    wcomb_d = nc.dram_tensor("wcomb", [8, T * 104], BF16,
                             kind="ExternalInput")
    out_d = nc.dram_tensor("out", [DIM, NP], F32, kind="ExternalOutput")

    with TileContext(nc) as tc:
        with (
            tc.tile_pool(name="const", bufs=1) as cpool,
            tc.tile_pool(name="adma", bufs=3) as apool,
            tc.tile_pool(name="work", bufs=2) as wpool,
            tc.tile_pool(name="pagg", bufs=1, space="PSUM") as pagg,
            tc.tile_pool(name="pbc", bufs=1, space="PSUM") as pbc,
            tc.tile_pool(name="pp1", bufs=3, space="PSUM") as pp1,
            tc.tile_pool(name="pcm", bufs=2, space="PSUM") as pcm,
        ):
            # xe first, then a-tiles, on the SP HWDGE ring (critical path);
            # remaining constant loads go on the ACT HWDGE ring.
            xe_t = cpool.tile([KT, NKT, XW], BF16)
            nc.sync.dma_start(out=xe_t, in_=xe_d[:, :, :])
            a_tiles = {}
            for h in range(2):
                for g in range(NG):
                    a_t = apool.tile([KT, KG, NH], BF16, tag="a")
                    nc.sync.dma_start(
                        out=a_t, in_=a_d[h, :, g * KG:(g + 1) * KG, :])
                    a_tiles[(h, g)] = a_t
            # dtprev: rows 0:96 = dataT (static), rows 96:104 = prev state
            dtprev_t = cpool.tile([104, NP], BF16)
            nc.scalar.dma_start(out=dtprev_t[0:DIM, :], in_=dt_d[:, :])
            nc.vector.memset(dtprev_t[DIM:104, :], 0.0)
            pt_t = cpool.tile([8, T, NP], F32)
            nc.scalar.dma_start(out=pt_t, in_=pt_d[:, :, :])
            wrp_t = cpool.tile([104, DIM], BF16)
            nc.scalar.dma_start(out=wrp_t, in_=wrp_d[:, :])
            wagg_t = cpool.tile([DIM, DIM], BF16)
            nc.scalar.dma_start(out=wagg_t, in_=wagg_d[:, :])
            wcomb_t = cpool.tile([8, T * 104], BF16)
            nc.scalar.dma_start(out=wcomb_t, in_=wcomb_d[:, :])

            ones_t = cpool.tile([1, DIM], F32)
            nc.vector.memset(ones_t, 1.0)
            h2_t = cpool.tile([8, T, NP], BF16)
            aggs_t = cpool.tile([DIM, NP], BF16)
            outt_t = cpool.tile([DIM, NP], F32)

            # phase 1: aggT[97, NH] = [X|1]^T @ adjT_shard  per node half
            for h in range(2):
                cs = slice(h * NH, (h + 1) * NH)
                aggp = pagg.tile([XW, NH], F32, tag="aggp")
                for g in range(NG):
                    a_t = a_tiles[(h, g)]
                    for j in range(KG):
                        k = g * KG + j
                        nc.tensor.matmul(aggp, xe_t[:, k, :], a_t[:, j, :],
                                         start=(k == 0), stop=(k == NKT - 1))
                # row-normalization: broadcast max(deg,1) to 96 partitions
                # (tiny matmul), then reciprocal on the wide tile (parallel
                # partitions -> ~5x faster than on [1, NH]), then scale.
                degm = wpool.tile([1, NH], F32, tag="degm")
                nc.vector.tensor_scalar_max(degm, aggp[DIM:DIM + 1, :], 1.0)
                rb_p = pbc.tile([DIM, NH], F32, tag="rbp")
                nc.tensor.matmul(rb_p, ones_t, degm, start=True, stop=True)
                rb_s = wpool.tile([DIM, NH], F32, tag="rbs")
                nc.vector.reciprocal(rb_s, rb_p)
                nc.vector.tensor_mul(aggs_t[:, cs], aggp[0:DIM, :], rb_s)

            # phase 2: sequential t-chain, both node halves interleaved.
            # Per t and half: 3 matmuls —
            #   p1 = w_rp[t]^T @ [dt; prev]  (chain) + wagg[t]^T @ aggs
            #   pcomb += wcomb[t]^T @ h(t)   (rows 0:96 = final acc,
            #                                 rows 96:104 = prev-update acc)
            pcombs = [pcm.tile([104, NH], F32, tag="pcm", name=f"pcomb{h}")
                      for h in range(2)]
            for t in range(T):
                r8 = slice(t * 8, t * 8 + 8)
                for h in range(2):
                    cs = slice(h * NH, (h + 1) * NH)
                    pcomb = pcombs[h]
                    p1 = pp1.tile([8, NH], F32, tag="p1")
                    nc.tensor.matmul(p1, wagg_t[:, r8], aggs_t[:, cs],
                                     start=True, stop=False)
                    nc.tensor.matmul(p1, wrp_t[:, r8], dtprev_t[:, cs],
                                     start=False, stop=True)
                    # h(t) = relu(p1) + pos(t)   (fused on DVE)
                    nc.vector.scalar_tensor_tensor(
                        h2_t[:, t, cs], p1, 0.0, pt_t[:, t, cs],
                        op0=mybir.AluOpType.max, op1=mybir.AluOpType.add)
                    # pcomb += wcomb[t]^T @ h(t)
                    nc.tensor.matmul(pcomb,
                                     wcomb_t[:, t * 104:(t + 1) * 104],
                                     h2_t[:, t, cs],
                                     start=(t == 0), stop=(t == T - 1),
                                     skip_group_check=True)
                    # prev = relu(p2 rows)  (ScalarE, off the DVE)
                    if t < T - 1:
                        nc.scalar.activation(
                            dtprev_t[DIM:104, cs], pcomb[DIM:104, :],
                            mybir.ActivationFunctionType.Relu)
            # final: out = relu(pcomb rows 0:96)
            for h in range(2):
                cs = slice(h * NH, (h + 1) * NH)
                nc.scalar.activation(outt_t[:, cs], pcombs[h][0:DIM, :],
                                     mybir.ActivationFunctionType.Relu)
                nc.sync.dma_start(out=out_d[:, cs], in_=outt_t[:, cs])

    split_multi_waits(nc)
    return nc


def prep_in_maps(adj, data, pos, his_W, cur_W, his_weight, cur_weight,
                 final_weight):
    adj = np.asarray(adj, dtype=np.float32)
    data = np.asarray(data, dtype=np.float32)
    pos = np.asarray(pos, dtype=np.float32)
    his_W = np.asarray(his_W, dtype=np.float32)
    cur_W = np.asarray(cur_W, dtype=np.float32)
    his_weight = np.asarray(his_weight, dtype=np.float32)
    cur_weight = np.asarray(cur_weight, dtype=np.float32)
    final_weight = np.asarray(final_weight, dtype=np.float32)

    # X = data rearranged [N, 96] (col = t*8+d), plus ones column;
    # contraction dim zero-padded to NK=5120 for full-128-partition tiles
    X = np.ascontiguousarray(data.transpose(1, 0, 2).reshape(N, DIM))
    Xe = np.zeros((NK, XW), np.float32)
    Xe[:N, :DIM] = X
    Xe[:N, DIM] = 1.0
    # pre-tiled for DMA: xe[p, k, c] = Xe[k*KT+p, c]
    xe_h = np.ascontiguousarray(
        Xe.reshape(NKT, KT, XW).transpose(1, 0, 2)).astype(BF16_NP)

    adjT = np.ascontiguousarray(adj.T).astype(BF16_NP)

    # weight packing (zero-padded block maps, see build_nc layout)
    wraw = np.zeros((DIM, DIM), np.float32)
    wagg = np.zeros((DIM, DIM), np.float32)
    wprev = np.zeros((8, DIM), np.float32)
    for t in range(T):
        wraw[t * 8:t * 8 + 7, t * 8:t * 8 + 7] = his_W[t][:, 0:7].T
        wraw[t * 8 + 7, t * 8 + 7] = cur_W[t][0, 0]
        wagg[t * 8:t * 8 + 7, t * 8:t * 8 + 7] = his_W[t][:, 7:14].T
        wagg[t * 8 + 7, t * 8 + 7] = cur_W[t][0, 1]
        wprev[0:7, t * 8:t * 8 + 7] = his_W[t][:, 21:28].T
        wprev[7, t * 8 + 7] = cur_W[t][0, 3]
    # w2s[d, 8t'+o] = prev-update weight from h(t') feature d to output o;
    # t-invariant blocks, accumulated incrementally on-chip.
    w2 = np.zeros((8, DIM), np.float32)
    for tp in range(T):
        w2[0:7, tp * 8:tp * 8 + 7] = his_weight[:, 7 * tp:7 * tp + 7].T
        w2[7, tp * 8 + 7] = cur_weight[0, tp]
    # interleaved feature (8t+d) -> reference feature (7t+d | 84+t)
    f_ref = np.array([7 * t + d if d < 7 else 84 + t
                      for t in range(T) for d in range(8)])
    wf96 = final_weight[:, f_ref].T  # [96 (8t+d), 96 (out)]
    # wf3[d, t*96+o] = wf96[8t+d, o]
    wf = np.ascontiguousarray(
        wf96.reshape(T, 8, DIM).transpose(1, 0, 2).reshape(8, T * DIM))
    # merged lhsT blocks:
    # wrp [104, 96]: rows 0:96 = wraw block-diag, rows 96:104 = wprev
    wrp = np.concatenate([wraw, wprev], axis=0)
    # wcomb [8, T*104]: per t, cols 0:96 = wf block(t), cols 96:104 = w2s(t)
    wcomb = np.zeros((8, T, 104), np.float32)
    for t in range(T):
        wcomb[:, t, 0:DIM] = wf[:, t * DIM:(t + 1) * DIM]
        wcomb[:, t, DIM:104] = w2[:, t * 8:(t + 1) * 8]
    wcomb = np.ascontiguousarray(wcomb.reshape(8, T * 104))

    in_maps = []
    for c in range(NCORES):
        c0 = c * NPC
        ac = np.zeros((NK, NP), BF16_NP)
        ac[:N, :NPC] = adjT[:, c0:c0 + NPC]
        # a[h, p, k, n] = ac[k*KT+p, h*NH+n]
        ah = np.ascontiguousarray(
            ac.reshape(NKT, KT, 2, NH).transpose(2, 1, 0, 3))
        dtc = np.zeros((DIM, NP), np.float32)
        dtc[:, :NPC] = data[:, c0:c0 + NPC, :].transpose(0, 2, 1).reshape(
            DIM, NPC)
        ptc = np.zeros((8, T, NP), np.float32)
        ptc[:, :, :NPC] = pos[:, c0:c0 + NPC, :].transpose(2, 0, 1)
        in_maps.append({
            "a": ah, "xe": xe_h, "dt": dtc.astype(BF16_NP), "pt": ptc,
            "wrp": wrp.astype(BF16_NP), "wagg": wagg.astype(BF16_NP),
            "wcomb": wcomb.astype(BF16_NP),
        })
    return in_maps


def assemble(results):
    out = np.empty((N, DIM), np.float32)
    for c in range(NCORES):
        out[c * NPC:(c + 1) * NPC, :] = results[c]["out"][:, :NPC].T
    return out


_NC_CACHE = None


def get_nc():
    global _NC_CACHE
    if _NC_CACHE is None:
        _NC_CACHE = build_nc()
    return _NC_CACHE


def run_spmd(in_maps, **kwargs):
    nc = get_nc()
    return bass_utils.run_bass_kernel_spmd(
        nc, in_maps, list(range(NCORES)), **kwargs)


def kernel(**inputs):
    in_maps = prep_in_maps(**inputs)
    res = run_spmd(in_maps)
    return assemble(res.results)
